# revision 19
# baseline (speedup 1.0000x reference)
"""Trainium2 Bass kernel for nn_ConditionalModuleBGR (histogram binning + MLP).

Strategy: data-parallel over 8 NeuronCores, 2 images (6 slabs of [128, 8192])
per core. Per slab, the 64-bin histogram is computed as a bilinear form on the
Tensor engine (radix 8x8):

  - hi = rne(8*x - 0.5) via exact fp32 magic rounding; y = fp16(x - hi/8).
  - 7 "coarse" step planes  A_h = [x32 >= t_h]  (t_h = h/8, nudged up one
    fp32 ulp for odd h to match rne tie-to-even), plus ones plane h=0.
  - 7 "fine" step planes    B_l = [y >= l/64]   plus ones plane l=0.
  - PE accumulates P[16h+j, 16l+j'] += sum_p A_h(e[p,j]) * B_l(e[p,j'])
    over 512 chained matmuls per slab (16 element-columns per matmul).
    Same-element products live on the j==j' diagonal blocks.
  - Exact drain (no reduced-precision matmul touches the large counts):
    diag mask + free-dim reduce (DVE), l-difference (DVE), PE transpose
    (exact passthrough), j-reduce + h-difference (DVE), then a small DRAM
    roundtrip to lay the 64 bins out as MLP features.
  - MLP (192 -> 128 relu -> 32) on PE + ACT as in the reference.

Plane generation is split across engines to balance them: DVE does prep +
the 7 fine planes (fp16/bf16 4x mode), ACT does 3 coarse planes (sigmoid
step trick), GPSIMD does 4 coarse planes (is_ge).
"""

import numpy as np

import concourse.bacc as bacc
import concourse.mybir as mybir
import concourse.tile as tile
from concourse import bass_utils

N_CORES = 8
N_IMG = 16
IMG_PER_CORE = N_IMG // N_CORES  # 2
CH = 3
SLABS = IMG_PER_CORE * CH  # 6
P = 128
F = 8192  # 1024*1024 / 128
BINS = 64
FEAT = CH * BINS  # 192
HID = 128
OUT = 32
W1_N = FEAT * HID
B1_N = HID
W2_N = HID * OUT
B2_N = OUT
G_OFF = W1_N + B1_N + W2_N + B2_N
N_PARAMS = 28864

C = 1024            # columns per chunk
NCHK = F // C       # 8 chunks per slab
KPC = C // 16       # 64 matmuls per chunk

F32 = mybir.dt.float32
F16 = mybir.dt.float16
BF16 = mybir.dt.bfloat16
ALU = mybir.AluOpType
ACTF = mybir.ActivationFunctionType

# The harness reference (numpy inputs -> axon jax) bins via C-style
# truncation: bin = floor(64*x) clipped to [0, 63], boundaries at k/64.
# Coarse index: hi = floor(8x) = rne(8x - 0.5), obtained via fp16
# output-rounding magic: w16 = fp16(8x + 1535.5) is 1536 + rne(8x - 0.5)
# exactly (fp16 ulp is 1 on [1024, 2048)), then hic = (w16 - 1536) min 7.
W_BIAS = 1536.0 - 0.5
W_SUB = -1536.0

ACT_COARSE = (1, 2, 3, 4)   # A-planes on the scalar engine (from x32)
DVE_COARSE = (5, 6, 7)      # A-planes on DVE (integer compare on hic)


def _thr_h(h: int) -> float:
    """fp32 threshold for coarse plane h: [x >= h/8], exact in fp32."""
    return float(np.float32(h / 8.0))


def _consts_np() -> np.ndarray:
    """[128, 256] fp32: BD (diag-block mask) | I128 (transpose identity)."""
    BD = np.zeros((128, 128), np.float32)
    for h in range(8):
        for l in range(8):
            for j in range(16):
                BD[16 * h + j, 16 * l + j] = 1.0
    I128 = np.eye(128, dtype=np.float32)
    return np.concatenate([BD, I128], axis=1).astype(np.float32)


def _build():
    nc = bacc.Bacc("TRN2", target_bir_lowering=False, debug=False,
                   num_devices=N_CORES)
    img = nc.dram_tensor("img", [SLABS, P, F], F32, kind="ExternalInput")
    params = nc.dram_tensor("params", [N_PARAMS], F32, kind="ExternalInput")
    cdram = nc.dram_tensor("consts", [P, 256], F32, kind="ExternalInput")
    out = nc.dram_tensor("out", [IMG_PER_CORE, OUT], F32, kind="ExternalOutput")
    fdbg = nc.dram_tensor("fdbg", [FEAT, IMG_PER_CORE], F32,
                          kind="ExternalOutput")

    img_ap = img.ap()
    par_ap = params.ap()
    out_ap = out.ap()
    fdbg_ap = fdbg.ap()

    with tile.TileContext(nc) as tc:
        with (
            tc.tile_pool(name="work", bufs=1) as work,
            tc.tile_pool(name="psum", bufs=1, space="PSUM") as psum,
        ):
            cst = work.tile([P, 256], F32, tag="cst")
            nc.sync.dma_start(out=cst[:], in_=cdram.ap())
            BD = cst[:, 0:128]
            I128 = cst[:, 128:256]

            # --- persistent plane buffers (manual double-buffer) ---
            A_bufs = [work.tile([P, KPC, 128], BF16, tag=f"A{i}", name=f"A{i}")
                      for i in range(2)]
            B_bufs = [work.tile([P, 8, C], BF16, tag=f"B{i}", name=f"B{i}")
                      for i in range(2)]
            for i in range(2):
                nc.vector.memset(A_bufs[i][:, :, 0:16], 1.0)   # h=0 ones
                nc.vector.memset(B_bufs[i][:, 0, :], 1.0)      # l=0 ones
            x32_bufs = [work.tile([P, C], F32, tag=f"x32_{i}", name=f"x32_{i}")
                        for i in range(2)]
            w16_bufs = [work.tile([P, C], F16, tag=f"w16_{i}", name=f"w16_{i}")
                        for i in range(2)]
            hic_bufs = [work.tile([P, C], F16, tag=f"hic_{i}", name=f"hic_{i}")
                        for i in range(2)]
            y_bufs = [work.tile([P, C], F16, tag=f"y_{i}", name=f"y_{i}")
                      for i in range(2)]

            Pp = [psum.tile([P, 128], F32, tag=f"P{i}", name=f"P{i}")
                  for i in range(2)]
            Tps = psum.tile([8, 128], F32, tag="Tps")
            T2ps = psum.tile([8, 8], F32, tag="T2ps")

            # drain scratch
            Pm = work.tile([P, 128], F32, tag="Pm")
            R9 = work.tile([P, 9], F32, tag="R9")
            nc.vector.memset(R9[:, 8:9], 0.0)
            Rd = work.tile([P, 8], F32, tag="Rd")
            H9 = work.tile([8, 9], F32, tag="H9")
            nc.vector.memset(H9[:, 8:9], 0.0)
            Hd = work.tile([8, 8], F32, tag="Hd")
            # per-slab: the feature DMA reads these; a shared tile would
            # need a DMA-read -> DVE-write WAR dependency that the tile
            # scheduler does not reliably enforce
            Hd2s = [work.tile([8, 8], F32, tag=f"Hd2_{s}", name=f"Hd2_{s}")
                    for s in range(SLABS)]

            feat_a = work.tile([P, IMG_PER_CORE], F32, tag="feata")
            feat_b = work.tile([BINS, IMG_PER_CORE], F32, tag="featb")

            act_bias = work.tile([P, 8], F32, tag="act_bias")
            for h in ACT_COARSE:
                nc.gpsimd.memset(
                    act_bias[:, h:h + 1], -_thr_h(h) * float(2.0 ** 38))

            for s in range(SLABS):
                Ps = Pp[s % 2]
                for ck in range(NCHK):
                    par = ck % 2
                    A = A_bufs[par]
                    B = B_bufs[par]
                    x32 = x32_bufs[par]
                    w16 = w16_bufs[par]
                    hic = hic_bufs[par]
                    y = y_bufs[par]

                    # --- load fp32 chunk ---
                    nc.sync.dma_start(
                        out=x32[:], in_=img_ap[s, :, ck * C:(ck + 1) * C])

                    x32_v = x32.rearrange("p (k j) -> p k j", j=16)
                    # --- A planes h=1..4: ACT sigmoid step from x32 ---
                    for h in ACT_COARSE:
                        nc.scalar.activation(
                            out=A[:, :, 16 * h:16 * h + 16], in_=x32_v[:],
                            func=ACTF.Sigmoid,
                            scale=float(2.0 ** 38),
                            bias=act_bias[:, h:h + 1])

                    # --- prep on DVE ---
                    # w16 = 1536 + rne(8x - 0.4375) via fp16 output rounding
                    nc.vector.tensor_scalar(
                        out=w16[:], in0=x32[:], scalar1=8.0, scalar2=W_BIAS,
                        op0=ALU.mult, op1=ALU.add)
                    # hic = min(w16 - 1536, 7): the coarse bin index
                    nc.vector.tensor_scalar(
                        out=hic[:], in0=w16[:], scalar1=W_SUB, scalar2=7.0,
                        op0=ALU.add, op1=ALU.min)
                    # y = fp16(x - hic/8)  (fp32 internal: exact, then rne16)
                    nc.vector.scalar_tensor_tensor(
                        out=y[:], in0=hic[:], scalar=-0.125, in1=x32[:],
                        op0=ALU.mult, op1=ALU.add)

                    # --- A planes h=5..7: integer compare on hic (fp16 4x) ---
                    for h in DVE_COARSE:
                        nc.vector.tensor_scalar(
                            out=A[:, :, 16 * h:16 * h + 16],
                            in0=hic.rearrange("p (k j) -> p k j", j=16),
                            scalar1=float(h) - 0.5, scalar2=None,
                            op0=ALU.is_ge)

                    # --- B planes: [y >= l/64] on DVE (fp16 4x) ---
                    for l in range(1, 8):
                        nc.vector.tensor_scalar(
                            out=B[:, l, :], in0=y[:],
                            scalar1=float(l) / 64.0, scalar2=None,
                            op0=ALU.is_ge)

                    # --- PE: 64 chained matmuls into PSUM ---
                    for k in range(KPC):
                        nc.tensor.matmul(
                            Ps[:],
                            A[:, k, :],
                            B[:, :, 16 * k:16 * k + 16],
                            start=(ck == 0 and k == 0),
                            stop=(ck == NCHK - 1 and k == KPC - 1),
                        )

                # --- exact drain of slab s ---
                i_img, c_ch = divmod(s, CH)
                # diag mask, then sum over j' within each l block
                nc.vector.tensor_tensor(
                    out=Pm[:], in0=Ps[:], in1=BD, op=ALU.mult)
                nc.vector.tensor_reduce(
                    out=R9[:, 0:8],
                    in_=Pm.rearrange("p (l j) -> p l j", j=16),
                    axis=mybir.AxisListType.X, op=ALU.add)
                # fine (l) difference: cum[l] - cum[l+1]
                nc.vector.tensor_tensor(
                    out=Rd[:], in0=R9[:, 0:8], in1=R9[:, 1:9],
                    op=ALU.subtract)
                # transpose [16h+j, l] -> [l, 16h+j] (exact passthrough)
                nc.tensor.transpose(Tps[:], Rd[:], I128)
                # sum over j within each h block
                nc.vector.tensor_reduce(
                    out=H9[:, 0:8],
                    in_=Tps.rearrange("p (h j) -> p h j", j=16),
                    axis=mybir.AxisListType.X, op=ALU.add)
                # coarse (h) difference -> hist[l, h]
                nc.vector.tensor_tensor(
                    out=Hd[:], in0=H9[:, 0:8], in1=H9[:, 1:9],
                    op=ALU.subtract)
                # transpose to [h, l] so the feature DMA is a natural
                # (h outer, l inner) enumeration
                nc.tensor.transpose(T2ps[:], Hd[:], I128[0:8, 0:8])
                Hd2 = Hd2s[s]
                nc.vector.tensor_copy(out=Hd2[:], in_=T2ps[:])
                # feature column: partition p = 64*ch + 8*h + l
                if c_ch == 2:
                    dst = feat_b[:, i_img:i_img + 1]
                else:
                    dst = feat_a[64 * c_ch:64 * c_ch + 64, i_img:i_img + 1]
                nc.sync.dma_start(out=dst, in_=Hd2[:])

            # --- MLP weights from params ---
            w1a = work.tile([P, HID], F32, tag="w1a")
            w1b = work.tile([FEAT - P, HID], F32, tag="w1b")
            nc.sync.dma_start(
                out=w1a[:], in_=par_ap[0:P * HID].rearrange("(a b) -> a b", a=P))
            nc.sync.dma_start(
                out=w1b[:],
                in_=par_ap[P * HID:W1_N].rearrange("(a b) -> a b", a=FEAT - P))
            b1 = work.tile([HID, 1], F32, tag="b1")
            nc.sync.dma_start(
                out=b1[:], in_=par_ap[W1_N:W1_N + B1_N].rearrange(
                    "(a b) -> a b", a=HID))
            w2 = work.tile([HID, OUT], F32, tag="w2")
            nc.sync.dma_start(
                out=w2[:],
                in_=par_ap[W1_N + B1_N:W1_N + B1_N + W2_N].rearrange(
                    "(a b) -> a b", a=HID))
            b2 = work.tile([OUT, 1], F32, tag="b2")
            nc.sync.dma_start(
                out=b2[:],
                in_=par_ap[W1_N + B1_N + W2_N:G_OFF].rearrange(
                    "(a b) -> a b", a=OUT))
            gsc = work.tile([1, 1], F32, tag="gsc")
            nc.sync.dma_start(
                out=gsc[:], in_=par_ap[G_OFF:G_OFF + 1].rearrange(
                    "(a b) -> a b", a=1))
            ones_out = work.tile([1, OUT], F32, tag="ones_out")
            nc.vector.memset(ones_out[:], 1.0)

            g_psum = psum.tile([OUT, 1], F32, tag="gpsum")
            nc.tensor.matmul(g_psum[:], ones_out[:], gsc[:], start=True,
                             stop=True)
            bias2 = work.tile([OUT, 1], F32, tag="bias2")
            nc.vector.tensor_add(out=bias2[:], in0=b2[:], in1=g_psum[:])

            nc.sync.dma_start(out=fdbg_ap[0:P, :], in_=feat_a[:])
            nc.sync.dma_start(out=fdbg_ap[P:FEAT, :], in_=feat_b[:])

            # --- layer 1: h = relu(w1.T @ feat + b1) (transposed) ---
            h_psum = psum.tile([HID, IMG_PER_CORE], F32, tag="hpsum")
            nc.tensor.matmul(h_psum[:], w1a[:], feat_a[:], start=True,
                             stop=False)
            nc.tensor.matmul(h_psum[:], w1b[:], feat_b[:], start=False,
                             stop=True)
            hmlp = work.tile([HID, IMG_PER_CORE], F32, tag="hmlp")
            nc.scalar.activation(
                out=hmlp[:], in_=h_psum[:], func=ACTF.Relu, bias=b1[:],
                scale=1.0)

            # --- layer 2: o = sigmoid(w2.T @ h + b2 + g) ---
            o_psum = psum.tile([OUT, IMG_PER_CORE], F32, tag="opsum")
            nc.tensor.matmul(o_psum[:], w2[:], hmlp[:], start=True, stop=True)
            o = work.tile([OUT, IMG_PER_CORE], F32, tag="o")
            nc.scalar.activation(
                out=o[:], in_=o_psum[:], func=ACTF.Sigmoid, bias=bias2[:],
                scale=1.0)

            nc.sync.dma_start(out=out_ap.rearrange("a b -> b a"), in_=o[:])

    nc.compile()
    return nc


_NC_CACHE = {}


def _get_nc():
    if "nc" not in _NC_CACHE:
        _NC_CACHE["nc"] = _build()
    return _NC_CACHE["nc"]


def make_in_maps(img: np.ndarray, params: np.ndarray):
    shards = img.reshape(N_CORES, SLABS, P, F)
    cst = _consts_np()
    return [
        {"img": shards[c], "params": params, "consts": cst}
        for c in range(N_CORES)
    ]


def kernel(img: np.ndarray, params: np.ndarray) -> np.ndarray:
    img = np.ascontiguousarray(img, dtype=np.float32)
    params = np.ascontiguousarray(params, dtype=np.float32)
    assert img.shape == (N_IMG, CH, 1024, 1024)
    assert params.shape == (N_PARAMS,)

    nc = _get_nc()
    in_maps = make_in_maps(img, params)
    res = bass_utils.run_bass_kernel_spmd(nc, in_maps,
                                          core_ids=list(range(N_CORES)))
    return np.concatenate([res.results[c]["out"] for c in range(N_CORES)],
                          axis=0)


# revision 20
# speedup vs baseline: 1.1503x; 1.1503x over previous
"""Trainium2 Bass kernel for nn_ConditionalModuleBGR (histogram binning + MLP).

Strategy: data-parallel over 8 NeuronCores, 2 images (6 slabs of [128, 8192])
per core. Per slab, the 64-bin histogram is computed as a bilinear form on the
Tensor engine (radix 8x8):

  - hi = rne(8*x - 0.5) via exact fp32 magic rounding; y = fp16(x - hi/8).
  - 7 "coarse" step planes  A_h = [x32 >= t_h]  (t_h = h/8, nudged up one
    fp32 ulp for odd h to match rne tie-to-even), plus ones plane h=0.
  - 7 "fine" step planes    B_l = [y >= l/64]   plus ones plane l=0.
  - PE accumulates P[16h+j, 16l+j'] += sum_p A_h(e[p,j]) * B_l(e[p,j'])
    over 512 chained matmuls per slab (16 element-columns per matmul).
    Same-element products live on the j==j' diagonal blocks.
  - Exact drain (no reduced-precision matmul touches the large counts):
    diag mask + free-dim reduce (DVE), l-difference (DVE), PE transpose
    (exact passthrough), j-reduce + h-difference (DVE), then a small DRAM
    roundtrip to lay the 64 bins out as MLP features.
  - MLP (192 -> 128 relu -> 32) on PE + ACT as in the reference.

Plane generation is split across engines to balance them: DVE does prep +
the 7 fine planes (fp16/bf16 4x mode), ACT does 3 coarse planes (sigmoid
step trick), GPSIMD does 4 coarse planes (is_ge).
"""

import numpy as np

import concourse.bacc as bacc
import concourse.mybir as mybir
import concourse.tile as tile
from concourse import bass_utils

N_CORES = 8
N_IMG = 16
IMG_PER_CORE = N_IMG // N_CORES  # 2
CH = 3
SLABS = IMG_PER_CORE * CH  # 6
P = 128
F = 8192  # 1024*1024 / 128
BINS = 64
FEAT = CH * BINS  # 192
HID = 128
OUT = 32
W1_N = FEAT * HID
B1_N = HID
W2_N = HID * OUT
B2_N = OUT
G_OFF = W1_N + B1_N + W2_N + B2_N
N_PARAMS = 28864

C = 2048            # columns per chunk
NCHK = F // C       # 4 chunks per slab
KPC = C // 16       # 128 matmuls per chunk

F32 = mybir.dt.float32
F16 = mybir.dt.float16
BF16 = mybir.dt.bfloat16
ALU = mybir.AluOpType
ACTF = mybir.ActivationFunctionType

# The harness reference (numpy inputs -> axon jax) bins via C-style
# truncation: bin = floor(64*x) clipped to [0, 63], boundaries at k/64.
# Coarse index: hi = floor(8x) = rne(8x - 0.5), obtained via fp16
# output-rounding magic: w16 = fp16(8x + 1535.5) is 1536 + rne(8x - 0.5)
# exactly (fp16 ulp is 1 on [1024, 2048)), then hic = (w16 - 1536) min 7.
W_BIAS = 1536.0 - 0.5
W_SUB = -1536.0

ACT_COARSE = (1, 2, 3, 4)   # A-planes on the scalar engine (from x32)
DVE_COARSE = (5, 6, 7)      # A-planes on DVE (integer compare on hic)


def _thr_h(h: int) -> float:
    """fp32 threshold for coarse plane h: [x >= h/8], exact in fp32."""
    return float(np.float32(h / 8.0))


def _consts_np() -> np.ndarray:
    """[128, 256] fp32: BD (diag-block mask) | I128 (transpose identity)."""
    BD = np.zeros((128, 128), np.float32)
    for h in range(8):
        for l in range(8):
            for j in range(16):
                BD[16 * h + j, 16 * l + j] = 1.0
    I128 = np.eye(128, dtype=np.float32)
    return np.concatenate([BD, I128], axis=1).astype(np.float32)


def _build():
    nc = bacc.Bacc("TRN2", target_bir_lowering=False, debug=False,
                   num_devices=N_CORES)
    img = nc.dram_tensor("img", [SLABS, P, F], F32, kind="ExternalInput")
    params = nc.dram_tensor("params", [N_PARAMS], F32, kind="ExternalInput")
    cdram = nc.dram_tensor("consts", [P, 256], F32, kind="ExternalInput")
    out = nc.dram_tensor("out", [IMG_PER_CORE, OUT], F32, kind="ExternalOutput")
    fdbg = nc.dram_tensor("fdbg", [FEAT, IMG_PER_CORE], F32,
                          kind="ExternalOutput")

    img_ap = img.ap()
    par_ap = params.ap()
    out_ap = out.ap()
    fdbg_ap = fdbg.ap()

    with tile.TileContext(nc) as tc:
        with (
            tc.tile_pool(name="work", bufs=1) as work,
            tc.tile_pool(name="psum", bufs=1, space="PSUM") as psum,
        ):
            cst = work.tile([P, 256], F32, tag="cst")
            nc.sync.dma_start(out=cst[:], in_=cdram.ap())
            BD = cst[:, 0:128]
            I128 = cst[:, 128:256]

            # --- persistent plane buffers (manual double-buffer) ---
            A_bufs = [work.tile([P, KPC, 128], BF16, tag=f"A{i}", name=f"A{i}")
                      for i in range(2)]
            B_bufs = [work.tile([P, 8, C], BF16, tag=f"B{i}", name=f"B{i}")
                      for i in range(2)]
            for i in range(2):
                nc.vector.memset(A_bufs[i][:, :, 0:16], 1.0)   # h=0 ones
                nc.vector.memset(B_bufs[i][:, 0, :], 1.0)      # l=0 ones
            x32_bufs = [work.tile([P, C], F32, tag=f"x32_{i}", name=f"x32_{i}")
                        for i in range(2)]
            w16_bufs = [work.tile([P, C], F16, tag=f"w16_{i}", name=f"w16_{i}")
                        for i in range(2)]
            hic_bufs = [work.tile([P, C], F16, tag=f"hic_{i}", name=f"hic_{i}")
                        for i in range(2)]
            y_bufs = [work.tile([P, C], F16, tag=f"y_{i}", name=f"y_{i}")
                      for i in range(2)]

            Pp = [psum.tile([P, 128], F32, tag=f"P{i}", name=f"P{i}")
                  for i in range(2)]
            Tps = psum.tile([8, 128], F32, tag="Tps")
            T2ps = psum.tile([8, 8], F32, tag="T2ps")

            # drain scratch
            Pm = work.tile([P, 128], F32, tag="Pm")
            R9 = work.tile([P, 9], F32, tag="R9")
            nc.vector.memset(R9[:, 8:9], 0.0)
            Rd = work.tile([P, 8], F32, tag="Rd")
            H9 = work.tile([8, 9], F32, tag="H9")
            nc.vector.memset(H9[:, 8:9], 0.0)
            Hd = work.tile([8, 8], F32, tag="Hd")
            # per-slab: the feature DMA reads these; a shared tile would
            # need a DMA-read -> DVE-write WAR dependency that the tile
            # scheduler does not reliably enforce
            Hd2s = [work.tile([8, 8], F32, tag=f"Hd2_{s}", name=f"Hd2_{s}")
                    for s in range(SLABS)]

            feat_a = work.tile([P, IMG_PER_CORE], F32, tag="feata")
            feat_b = work.tile([BINS, IMG_PER_CORE], F32, tag="featb")

            act_bias = work.tile([P, 8], F32, tag="act_bias")
            for h in ACT_COARSE:
                nc.gpsimd.memset(
                    act_bias[:, h:h + 1], -_thr_h(h) * float(2.0 ** 38))

            for s in range(SLABS):
                Ps = Pp[s % 2]
                for ck in range(NCHK):
                    par = ck % 2
                    A = A_bufs[par]
                    B = B_bufs[par]
                    x32 = x32_bufs[par]
                    w16 = w16_bufs[par]
                    hic = hic_bufs[par]
                    y = y_bufs[par]

                    # --- load fp32 chunk ---
                    nc.sync.dma_start(
                        out=x32[:], in_=img_ap[s, :, ck * C:(ck + 1) * C])

                    x32_v = x32.rearrange("p (k j) -> p k j", j=16)
                    # --- A planes h=1..4: ACT sigmoid step from x32 ---
                    for h in ACT_COARSE:
                        nc.scalar.activation(
                            out=A[:, :, 16 * h:16 * h + 16], in_=x32_v[:],
                            func=ACTF.Sigmoid,
                            scale=float(2.0 ** 38),
                            bias=act_bias[:, h:h + 1])

                    # --- prep on DVE ---
                    # w16 = 1536 + rne(8x - 0.4375) via fp16 output rounding
                    nc.vector.tensor_scalar(
                        out=w16[:], in0=x32[:], scalar1=8.0, scalar2=W_BIAS,
                        op0=ALU.mult, op1=ALU.add)
                    # hic = min(w16 - 1536, 7): the coarse bin index
                    nc.vector.tensor_scalar(
                        out=hic[:], in0=w16[:], scalar1=W_SUB, scalar2=7.0,
                        op0=ALU.add, op1=ALU.min)
                    # y = fp16(x - hic/8)  (fp32 internal: exact, then rne16)
                    nc.vector.scalar_tensor_tensor(
                        out=y[:], in0=hic[:], scalar=-0.125, in1=x32[:],
                        op0=ALU.mult, op1=ALU.add)

                    # --- A planes h=5..7: integer compare on hic (fp16 4x) ---
                    for h in DVE_COARSE:
                        nc.vector.tensor_scalar(
                            out=A[:, :, 16 * h:16 * h + 16],
                            in0=hic.rearrange("p (k j) -> p k j", j=16),
                            scalar1=float(h) - 0.5, scalar2=None,
                            op0=ALU.is_ge)

                    # --- B planes: [y >= l/64] on DVE (fp16 4x) ---
                    for l in range(1, 8):
                        nc.vector.tensor_scalar(
                            out=B[:, l, :], in0=y[:],
                            scalar1=float(l) / 64.0, scalar2=None,
                            op0=ALU.is_ge)

                    # --- PE: 64 chained matmuls into PSUM ---
                    for k in range(KPC):
                        nc.tensor.matmul(
                            Ps[:],
                            A[:, k, :],
                            B[:, :, 16 * k:16 * k + 16],
                            start=(ck == 0 and k == 0),
                            stop=(ck == NCHK - 1 and k == KPC - 1),
                        )

                # --- exact drain of slab s ---
                i_img, c_ch = divmod(s, CH)
                # diag mask, then sum over j' within each l block
                nc.vector.tensor_tensor(
                    out=Pm[:], in0=Ps[:], in1=BD, op=ALU.mult)
                nc.vector.tensor_reduce(
                    out=R9[:, 0:8],
                    in_=Pm.rearrange("p (l j) -> p l j", j=16),
                    axis=mybir.AxisListType.X, op=ALU.add)
                # fine (l) difference: cum[l] - cum[l+1]
                nc.vector.tensor_tensor(
                    out=Rd[:], in0=R9[:, 0:8], in1=R9[:, 1:9],
                    op=ALU.subtract)
                # transpose [16h+j, l] -> [l, 16h+j] (exact passthrough)
                nc.tensor.transpose(Tps[:], Rd[:], I128)
                # sum over j within each h block
                nc.vector.tensor_reduce(
                    out=H9[:, 0:8],
                    in_=Tps.rearrange("p (h j) -> p h j", j=16),
                    axis=mybir.AxisListType.X, op=ALU.add)
                # coarse (h) difference -> hist[l, h]
                nc.vector.tensor_tensor(
                    out=Hd[:], in0=H9[:, 0:8], in1=H9[:, 1:9],
                    op=ALU.subtract)
                # transpose to [h, l] so the feature DMA is a natural
                # (h outer, l inner) enumeration
                nc.tensor.transpose(T2ps[:], Hd[:], I128[0:8, 0:8])
                Hd2 = Hd2s[s]
                nc.vector.tensor_copy(out=Hd2[:], in_=T2ps[:])
                # feature column: partition p = 64*ch + 8*h + l
                if c_ch == 2:
                    dst = feat_b[:, i_img:i_img + 1]
                else:
                    dst = feat_a[64 * c_ch:64 * c_ch + 64, i_img:i_img + 1]
                nc.sync.dma_start(out=dst, in_=Hd2[:])

            # --- MLP weights from params ---
            w1a = work.tile([P, HID], F32, tag="w1a")
            w1b = work.tile([FEAT - P, HID], F32, tag="w1b")
            nc.sync.dma_start(
                out=w1a[:], in_=par_ap[0:P * HID].rearrange("(a b) -> a b", a=P))
            nc.sync.dma_start(
                out=w1b[:],
                in_=par_ap[P * HID:W1_N].rearrange("(a b) -> a b", a=FEAT - P))
            b1 = work.tile([HID, 1], F32, tag="b1")
            nc.sync.dma_start(
                out=b1[:], in_=par_ap[W1_N:W1_N + B1_N].rearrange(
                    "(a b) -> a b", a=HID))
            w2 = work.tile([HID, OUT], F32, tag="w2")
            nc.sync.dma_start(
                out=w2[:],
                in_=par_ap[W1_N + B1_N:W1_N + B1_N + W2_N].rearrange(
                    "(a b) -> a b", a=HID))
            b2 = work.tile([OUT, 1], F32, tag="b2")
            nc.sync.dma_start(
                out=b2[:],
                in_=par_ap[W1_N + B1_N + W2_N:G_OFF].rearrange(
                    "(a b) -> a b", a=OUT))
            gsc = work.tile([1, 1], F32, tag="gsc")
            nc.sync.dma_start(
                out=gsc[:], in_=par_ap[G_OFF:G_OFF + 1].rearrange(
                    "(a b) -> a b", a=1))
            ones_out = work.tile([1, OUT], F32, tag="ones_out")
            nc.vector.memset(ones_out[:], 1.0)

            g_psum = psum.tile([OUT, 1], F32, tag="gpsum")
            nc.tensor.matmul(g_psum[:], ones_out[:], gsc[:], start=True,
                             stop=True)
            bias2 = work.tile([OUT, 1], F32, tag="bias2")
            nc.vector.tensor_add(out=bias2[:], in0=b2[:], in1=g_psum[:])

            nc.sync.dma_start(out=fdbg_ap[0:P, :], in_=feat_a[:])
            nc.sync.dma_start(out=fdbg_ap[P:FEAT, :], in_=feat_b[:])

            # --- layer 1: h = relu(w1.T @ feat + b1) (transposed) ---
            h_psum = psum.tile([HID, IMG_PER_CORE], F32, tag="hpsum")
            nc.tensor.matmul(h_psum[:], w1a[:], feat_a[:], start=True,
                             stop=False)
            nc.tensor.matmul(h_psum[:], w1b[:], feat_b[:], start=False,
                             stop=True)
            hmlp = work.tile([HID, IMG_PER_CORE], F32, tag="hmlp")
            nc.scalar.activation(
                out=hmlp[:], in_=h_psum[:], func=ACTF.Relu, bias=b1[:],
                scale=1.0)

            # --- layer 2: o = sigmoid(w2.T @ h + b2 + g) ---
            o_psum = psum.tile([OUT, IMG_PER_CORE], F32, tag="opsum")
            nc.tensor.matmul(o_psum[:], w2[:], hmlp[:], start=True, stop=True)
            o = work.tile([OUT, IMG_PER_CORE], F32, tag="o")
            nc.scalar.activation(
                out=o[:], in_=o_psum[:], func=ACTF.Sigmoid, bias=bias2[:],
                scale=1.0)

            nc.sync.dma_start(out=out_ap.rearrange("a b -> b a"), in_=o[:])

    nc.compile()
    return nc


_NC_CACHE = {}


def _get_nc():
    if "nc" not in _NC_CACHE:
        _NC_CACHE["nc"] = _build()
    return _NC_CACHE["nc"]


def make_in_maps(img: np.ndarray, params: np.ndarray):
    shards = img.reshape(N_CORES, SLABS, P, F)
    cst = _consts_np()
    return [
        {"img": shards[c], "params": params, "consts": cst}
        for c in range(N_CORES)
    ]


def kernel(img: np.ndarray, params: np.ndarray) -> np.ndarray:
    img = np.ascontiguousarray(img, dtype=np.float32)
    params = np.ascontiguousarray(params, dtype=np.float32)
    assert img.shape == (N_IMG, CH, 1024, 1024)
    assert params.shape == (N_PARAMS,)

    nc = _get_nc()
    in_maps = make_in_maps(img, params)
    res = bass_utils.run_bass_kernel_spmd(nc, in_maps,
                                          core_ids=list(range(N_CORES)))
    return np.concatenate([res.results[c]["out"] for c in range(N_CORES)],
                          axis=0)
